# revision 1
# baseline (speedup 1.0000x reference)
"""Additive (Bahdanau) attention kernel for Trainium2, 8 NeuronCores.

score[b,tq,tk] = sum_a w3[a] * tanh( (Q@W1.T)[b,tq,a] + (K@W2.T)[b,tk,a] )
out = softmax(where(mask, score, -1e10), axis=tk)

Sharding: data-parallel over (b, tq-half): core = b*2 + half, each core owns
one batch element and 128 query rows; everything else is local.

Per-core dataflow (engines balanced against the ScalarE tanh roofline of
~109us/core = 16.8M elems / 128 lanes / 1.2GHz):
  - PE projects qp[a,tq] and kp[a,tk] (contraction over d, fp32r = 1 cyc/row;
    next chunk's weights/projections prefetched so the pipeline never drains).
  - DVE broadcast-adds qp columns onto kp (tensor_scalar, fp32 2x mode, ~99us).
  - ACT runs one fused tanh per 16-tq group ([128, 4096] instructions to
    amortize the ~224-cycle instruction init; ~115us busy, the critical path).
  - PE reduces over `a` with a sliding one-hot w3 window [128, 32] as the
    stationary operand: matmul row m carries w3 iff m == tq%32, so each tq's
    score row accumulates at PSUM partition tq%32 of its 32-row strip bank
    (fp32r matmuls must write PSUM at partition 0, hence 4 strip banks).
  - Masked softmax: mask folded in as a precomputed additive (m-1)*1e10 term
    fused with the strip gather, exp via ACT with per-partition -rowmax bias
    and fused accum_out row-sum.
Inputs are staged host-side: transposed Q/K/W (so no on-device transposes),
w3 pre-padded into the sliding-window buffer, mask pre-converted to the
additive term. All DMAs are batched into single multi-dim-AP transfers
(the HWDGE queue serializes at ~0.6us per dma_start instruction).
"""

import numpy as np

import concourse.bass as bass
import concourse.bacc as bacc
import concourse.tile as tile
from concourse import mybir
from concourse.bass_utils import run_bass_kernel_spmd

B, TQ, TK, DQ, DK, A = 4, 256, 256, 1024, 1024, 512
NCORES = 8
TQH = TQ // 2  # tq rows per core
NCH = A // 128  # a-chunks
ND = DQ // 128  # d-chunks
GRP = 16  # tq rows per ACT group
PIPE_BUFS = 3  # S/H pool buffering depth
H_DT = None  # override h dtype (None -> F32R)
ABLATE = None  # None | 'nomm' | 'noadd' (timing experiments only; wrong results)
S_BF16 = False  # bf16 kp/S: DVE tensor_scalar 4x mode (~65us vs 99us), costs ~2e-3 rel err
NGRP = TQH // GRP

F32 = mybir.dt.float32
F32R = mybir.dt.float32r
BF16 = mybir.dt.bfloat16
TANH = mybir.ActivationFunctionType.Tanh
EXP = mybir.ActivationFunctionType.Exp
ADD = mybir.AluOpType.add
MAX = mybir.AluOpType.max
AXX = mybir.AxisListType.X


def _build(nc: bass.Bass, iters: int = 1):
    qt = nc.dram_tensor("qt", [DQ, TQ], F32R, kind="ExternalInput")
    kt = nc.dram_tensor("kt", [DK, TK], F32R, kind="ExternalInput")
    w1t = nc.dram_tensor("w1t", [DQ, A], F32R, kind="ExternalInput")
    w2t = nc.dram_tensor("w2t", [DK, A], F32R, kind="ExternalInput")
    w3p = nc.dram_tensor("w3p", [128, NCH * 64], F32R, kind="ExternalInput")
    madd = nc.dram_tensor("madd", [TQH, TK], F32, kind="ExternalInput")
    out = nc.dram_tensor("out", [TQH, TK], F32, kind="ExternalOutput")

    # single-DMA layouts: partition-major with d-tiles and a-chunks as free dims
    qt3 = qt.ap().rearrange("(n p) m -> p n m", p=128)
    kt3 = kt.ap().rearrange("(n p) m -> p n m", p=128)
    w1t4 = w1t.ap().rearrange("(n p) (c m) -> p c n m", p=128, c=NCH)
    w2t4 = w2t.ap().rearrange("(n p) (c m) -> p c n m", p=128, c=NCH)

    with tile.TileContext(nc) as tc:
      for _it in range(iters):
        with (
            tc.tile_pool(name="consts", bufs=1) as consts,
            tc.tile_pool(name="wpool", bufs=1) as wpool,
            tc.tile_pool(name="xpool", bufs=1) as xpool,
            tc.tile_pool(name="spool", bufs=PIPE_BUFS) as spool,
            tc.tile_pool(name="hpool", bufs=PIPE_BUFS) as hpool,
            tc.tile_pool(name="fin", bufs=1) as fin,
            tc.tile_pool(name="pproj", bufs=2, space="PSUM") as pproj,
            tc.tile_pool(name="pscore", bufs=1, space="PSUM") as pscore,
        ):
            # ---- load inputs ----
            # critical-path loads first: the HWDGE queue serializes dma_starts,
            # and the first projections need qts/kts (+ w1c/w2c emitted below).
            qts = xpool.tile([128, ND, TQ], F32R)
            nc.sync.dma_start(out=qts, in_=qt3)
            kts = xpool.tile([128, ND, TK], F32R)
            nc.sync.dma_start(out=kts, in_=kt3)
            qt_sb = [qts[:, d, :] for d in range(ND)]
            kt_sb = [kts[:, d, :] for d in range(ND)]

            qp_sb = consts.tile([128, NCH, TQ], F32)
            kp_sb = consts.tile([128, NCH, TK], BF16 if S_BF16 else F32)
            # fp32r matmuls must write PSUM at partition 0: one bank per
            # 32-row tq strip, gathered into SBUF afterwards.
            strip_ps = []
            for k in range(4):
                strip_k = pscore.tile([32, TK], F32, tag=f"strip{k}", name=f"strip{k}")
                strip_ps.append(strip_k)

            def emit_proj(c):
                # qp[a, tq] = sum_d W1[a, d] Q[tq, d]; lhsT = W1T slice [d, a]
                w1c = wpool.tile([128, ND, 128], F32R, tag="w1c", bufs=2, name=f"w1c{c}")
                w2c = wpool.tile([128, ND, 128], F32R, tag="w2c", bufs=2, name=f"w2c{c}")
                nc.sync.dma_start(out=w1c, in_=w1t4[:, c])
                nc.sync.dma_start(out=w2c, in_=w2t4[:, c])
                psq = pproj.tile([128, TQ], F32, tag="psq", name=f"psq{c}")
                for d in range(ND):
                    nc.tensor.matmul(
                        psq,
                        lhsT=w1c[:, d, :],
                        rhs=qt_sb[d],
                        start=(d == 0),
                        stop=(d == ND - 1),
                    )
                nc.vector.tensor_copy(qp_sb[:, c, :], psq)
                psk = pproj.tile([128, TK], F32, tag="psk", name=f"psk{c}")
                for d in range(ND):
                    nc.tensor.matmul(
                        psk,
                        lhsT=w2c[:, d, :],
                        rhs=kt_sb[d],
                        start=(d == 0),
                        stop=(d == ND - 1),
                    )
                nc.vector.tensor_copy(kp_sb[:, c, :], psk)

            emit_proj(0)
            w3p_sb = consts.tile([128, NCH, 64], F32R)
            nc.sync.dma_start(out=w3p_sb, in_=w3p.ap().rearrange("p (c j) -> p c j", c=NCH))
            if H_DT is not None:
                w3p_lhs = consts.tile([128, NCH, 64], H_DT)
                nc.vector.tensor_copy(w3p_lhs, w3p_sb.bitcast(F32))
            else:
                w3p_lhs = w3p_sb
            madd_sb = consts.tile([128, TK], F32)
            nc.sync.dma_start(out=madd_sb, in_=madd.ap())
            for c in range(NCH):
                # prefetch next chunk's projections so the DVE/ACT pipeline
                # never drains at the chunk boundary
                if c + 1 < NCH:
                    emit_proj(c + 1)

                # ---- main loop for chunk c ----
                for g in range(NGRP):
                    s_t = spool.tile([128, GRP, TK], BF16 if S_BF16 else F32, tag="s")
                    for j in range(GRP):
                        if ABLATE == "noadd" and j > 0:
                            continue
                        tq = g * GRP + j
                        nc.vector.tensor_scalar_add(
                            s_t[:, j, :], kp_sb[:, c, :], qp_sb[:, c, tq : tq + 1]
                        )
                    h_t = hpool.tile([128, GRP, TK], H_DT or F32R, tag="h")
                    nc.scalar.activation(h_t, s_t, TANH)
                    for j in range(GRP):
                        tq = g * GRP + j
                        k, jj = divmod(tq, 32)
                        if ABLATE == "nomm" and not (c == NCH - 1 and jj == 31):
                            continue
                        nc.tensor.matmul(
                            strip_ps[k],
                            lhsT=w3p_lhs[:, c, 32 - jj : 64 - jj],
                            rhs=h_t[:, j, :],
                            start=(c == 0 and jj == 0) or (ABLATE == "nomm"),
                            stop=(c == NCH - 1 and jj == 31),
                            skip_group_check=True,
                        )

            # ---- masked softmax over tk ----
            sc = fin.tile([128, TK], F32)
            for k in range(4):
                nc.vector.tensor_tensor(
                    sc[32 * k : 32 * (k + 1), :],
                    strip_ps[k],
                    madd_sb[32 * k : 32 * (k + 1), :],
                    op=ADD,
                )
            negmax = fin.tile([128, 1], F32)
            nc.vector.tensor_reduce(negmax, sc, axis=AXX, op=MAX, negate=True)
            e_t = fin.tile([128, TK], F32)
            denom = fin.tile([128, 1], F32)
            nc.scalar.activation(e_t, sc, EXP, bias=negmax, accum_out=denom)
            rden = fin.tile([128, 1], F32)
            nc.vector.reciprocal(rden, denom)
            out_sb = fin.tile([128, TK], F32)
            nc.vector.tensor_scalar_mul(out_sb, e_t, rden)
            nc.sync.dma_start(out=out.ap(), in_=out_sb)

    return nc


_NC_CACHE = None


def _get_nc():
    global _NC_CACHE
    if _NC_CACHE is None:
        nc = bacc.Bacc("TRN2", target_bir_lowering=False, debug=False, num_devices=NCORES)
        _build(nc)
        nc.compile()
        _NC_CACHE = nc
    return _NC_CACHE


def make_in_maps(Q, K, mask, W1, W2, w3):
    """Host-side sharding/layout prep. Returns one input dict per core."""
    Q = np.ascontiguousarray(np.asarray(Q, dtype=np.float32)).reshape(B, TQ, DQ)
    K = np.ascontiguousarray(np.asarray(K, dtype=np.float32)).reshape(B, TK, DK)
    mask = np.asarray(mask)
    W1 = np.asarray(W1, dtype=np.float32)
    W2 = np.asarray(W2, dtype=np.float32)
    w3 = np.asarray(w3, dtype=np.float32)

    w1t = np.ascontiguousarray(W1.T)  # [DQ, A]
    w2t = np.ascontiguousarray(W2.T)  # [DK, A]
    w3p = np.zeros((128, NCH, 64), np.float32)
    for c in range(NCH):
        w3p[:, c, 32] = w3[c * 128 : (c + 1) * 128]
    w3p = w3p.reshape(128, NCH * 64)
    madd_full = (mask.astype(np.float32) - 1.0) * 1e10  # [B, TQ, TK]

    in_maps = []
    for core in range(NCORES):
        b, half = divmod(core, 2)
        # core's own tq half first so the kernel can use fixed columns 0..127
        qrot = np.concatenate(
            [Q[b, half * TQH : (half + 1) * TQH], Q[b, (1 - half) * TQH : (2 - half) * TQH]],
            axis=0,
        )
        in_maps.append(
            {
                "qt": np.ascontiguousarray(qrot.T),  # [DQ, TQ]
                "kt": np.ascontiguousarray(K[b].T),  # [DK, TK]
                "w1t": w1t,
                "w2t": w2t,
                "w3p": w3p,
                "madd": np.ascontiguousarray(madd_full[b, half * TQH : (half + 1) * TQH]),
            }
        )
    return in_maps


def _gather(results):
    out = np.empty((B, TQ, TK), np.float32)
    for core in range(NCORES):
        b, half = divmod(core, 2)
        out[b, half * TQH : (half + 1) * TQH] = results[core]["out"]
    return out


def run(inputs, **kwargs):
    nc = _get_nc()
    in_maps = make_in_maps(**inputs)
    res = run_bass_kernel_spmd(nc, in_maps, core_ids=list(range(NCORES)), **kwargs)
    return _gather(res.results), res


def kernel(**inputs):
    out, _ = run(inputs)
    return out

